# revision 10
# baseline (speedup 1.0000x reference)
"""Fused cross-attention kernel for TRN2, sharded over 8 NeuronCores.

Sharding: core = 2*b + g  (b = batch 0..3 data-parallel, g = head-group 0..1
tensor-parallel over heads: heads g*8..g*8+7, i.e. columns g*512..(g+1)*512 of
Wq/Wk/Wv and rows g*512..(g+1)*512 of Wo). Each core computes a partial
out = softmax((x@Wq)(ctx@Wk)^T/sqrt(d)) (ctx@Wv) @ Wo_slice for its batch;
the host sums the two head-group partials per batch and adds bo.

Schedule: the ScalarE exp stream (256 x [128,1024] activations ~ 285us) is
the critical resource; everything else hides under it.  Loops run j (n-block)
outer, pair inner, m-tile innermost.  All non-attention PE work (K^T/V/Q
projections, out = O^T.T@Wo) is emitted through a work queue that drips ~1
matmul per t-step into the PE queue, plus watermark draining so the first
j-block can start ~15us in while K/V production continues underneath.
S pairs are row-tiled (heads at PE row tiles 0/64) and stream concurrently.
Mask is folded into V and the ones-column (masked rows contribute 0 to both
numerator and softmax sum), so exp needs no bias operand.  Normalize uses
DVE + a 0-stride DMA partition-broadcast (no gpsimd).
"""
import numpy as np

B, N, M = 4, 2048, 2048
DQ = 1024
DC = 1024
H = 16
DH = 64
INNER = 1024
HG = 2            # head groups (tensor parallel)
HPC = H // HG     # heads per core
CI = HPC * DH     # 512 inner dims per core
NCORES = 8
PT = 128          # partition tile
NB = 512          # n-block
KT_DQ = DQ // PT  # 8 contraction tiles for projections
MT = M // PT      # 16 m-tiles
NT = N // PT      # 16 n-tiles
NJ = N // NB      # 4 n-blocks
SCALE = DH ** -0.5

_CACHE = {}


class WorkQueue:
    """Ordered generators of background PE work, dripped into the emission
    stream.  pump(n) advances n yield-units; drain(tag) runs until the
    generator registered under tag has completed."""

    def __init__(self):
        self.items = []      # list of (tag, generator)
        self.done = set()
        self.active = None   # (tag, gen)

    def add(self, tag, gen):
        self.items.append((tag, gen))

    def _step(self):
        # advance the current generator by one unit; True if work remains
        if self.active is None:
            if not self.items:
                return False
            self.active = self.items.pop(0)
        tag, gen = self.active
        try:
            next(gen)
        except StopIteration:
            self.done.add(tag)
            self.active = None
        return True

    def pump(self, n):
        for _ in range(n):
            if not self._step():
                return

    def drain(self, tag):
        while tag not in self.done:
            if not self._step():
                raise RuntimeError(f"work item {tag} never registered")

    def drain_all(self):
        while self._step():
            pass


def _build_nc():
    import concourse.bass as bass
    import concourse.mybir as mybir
    import concourse.tile as tile
    from concourse import bacc

    F32 = mybir.dt.float32
    BF16 = mybir.dt.bfloat16
    EXP = mybir.ActivationFunctionType.Exp

    nc = bacc.Bacc("TRN2", target_bir_lowering=False, debug=False,
                   num_devices=NCORES)

    # host-prearranged: [128, ...] so every load is a 2D contiguous DMA
    xT_d = nc.dram_tensor("xT", [PT, NJ * KT_DQ * NB], BF16,
                          kind="ExternalInput")
    ctxT_d = nc.dram_tensor("ctxT", [PT, 4 * KT_DQ * NB], BF16,
                            kind="ExternalInput")
    wq_d = nc.dram_tensor("wq", [PT, KT_DQ * CI], BF16, kind="ExternalInput")
    wk_d = nc.dram_tensor("wk", [PT, KT_DQ * CI], BF16, kind="ExternalInput")
    wv_d = nc.dram_tensor("wv", [PT, KT_DQ * CI], BF16, kind="ExternalInput")
    wo_d = nc.dram_tensor("wo", [PT, 4 * INNER], BF16, kind="ExternalInput")
    mb_d = nc.dram_tensor("mask01", [PT, MT], F32, kind="ExternalInput")
    out_d = nc.dram_tensor("out", [N, INNER], F32, kind="ExternalOutput")

    with tile.TileContext(nc) as tc:
      with (
          tc.tile_pool(name="persist", bufs=1) as pp,
          tc.tile_pool(name="ctx_s", bufs=3) as pcs,
          tc.tile_pool(name="xt_s", bufs=2) as pxs,
          tc.tile_pool(name="pe_p", bufs=3) as ppe,
          tc.tile_pool(name="small", bufs=2) as psm,
          tc.tile_pool(name="stg", bufs=1) as pst,
          tc.tile_pool(name="dout", bufs=2) as pdo,
          tc.tile_pool(name="sps_p", bufs=2, space="PSUM") as sps_p,
          tc.tile_pool(name="oacc", bufs=1, space="PSUM") as oacc_p,
          tc.tile_pool(name="aux", bufs=2, space="PSUM") as aux_p,
      ):
        kt = [pp.tile([PT, M], BF16, tag=f"kt{p}", name=f"kt{p}")
              for p in range(4)]
        vt = [pp.tile([PT, HPC * (DH + 1)], BF16, tag=f"vt{t}", name=f"vt{t}")
              for t in range(MT)]
        qt = [pp.tile([PT, N], BF16, tag=f"qt{p}", name=f"qt{p}")
              for p in range(4)]
        ot = [pp.tile([PT, N], BF16, tag=f"ot{p}", name=f"ot{p}")
              for p in range(4)]
        mask_t = pp.tile([PT, MT], F32, tag="mask", name="mask")
        ones64 = pp.tile([1, DH], BF16, tag="ones64", name="ones64")
        wq_all = pp.tile([PT, KT_DQ * CI], BF16, tag="wq_all", name="wq_all")
        wk_all = pp.tile([PT, KT_DQ * CI], BF16, tag="wk_all", name="wk_all")
        wv_all = pp.tile([PT, KT_DQ * CI], BF16, tag="wv_all", name="wv_all")
        wo_all = pp.tile([PT, 4 * INNER], BF16, tag="wo_all", name="wo_all")
        wq_t = [wq_all[:, k * CI:(k + 1) * CI] for k in range(KT_DQ)]
        wk_t = [wk_all[:, k * CI:(k + 1) * CI] for k in range(KT_DQ)]
        wv_t = [wv_all[:, k * CI:(k + 1) * CI] for k in range(KT_DQ)]
        wo_t = [wo_all[:, k * INNER:(k + 1) * INNER] for k in range(4)]

        wq = WorkQueue()

        # ---------------- staging DMA helpers ----------------
        ctx_tiles = {}   # q -> list of 8 tiles

        def dma_ctx(q):
            ca = pcs.tile([PT, KT_DQ * NB], BF16, tag="ctxq", name="ctxq")
            nc.sync.dma_start(
                ca[:], ctxT_d[:, q * KT_DQ * NB:(q + 1) * KT_DQ * NB])
            ctx_tiles[q] = [ca[:, k * NB:(k + 1) * NB] for k in range(KT_DQ)]

        xt_tiles = {}    # j -> list of 8 tiles

        def dma_x(j):
            xa = pxs.tile([PT, KT_DQ * NB], BF16, tag="xq", name="xq")
            nc.sync.dma_start(
                xa[:], xT_d[:, j * KT_DQ * NB:(j + 1) * KT_DQ * NB])
            xt_tiles[j] = [xa[:, k * NB:(k + 1) * NB] for k in range(KT_DQ)]

        # ---------------- background work generators ----------------
        def g_fn(fn, *a):
            def g():
                fn(*a)
                yield
            return g()

        def g_ktq(p, q):
            ctx = ctx_tiles[q]
            ps = aux_p.tile([PT, NB], F32, tag="aux", name="aux")
            for k in range(KT_DQ):
                nc.tensor.matmul(ps[:], wk_t[k][:, p * PT:(p + 1) * PT],
                                 ctx[k],
                                 start=(k == 0), stop=(k == KT_DQ - 1))
                if k % 2 == 1:
                    yield
            nc.vector.tensor_copy(kt[p][:, q * NB:(q + 1) * NB], ps[:])
            yield

        def g_vt(t):
            q = t // 4
            ti = t % 4
            ctx = ctx_tiles[q]
            ps = aux_p.tile([PT, CI], F32, tag="aux", name="aux")
            for k in range(KT_DQ):
                nc.tensor.matmul(ps[:], ctx[k][:, ti * PT:(ti + 1) * PT],
                                 wv_t[k][:],
                                 start=(k == 0), stop=(k == KT_DQ - 1))
                if k % 2 == 1:
                    yield
            dst = vt[t][:].rearrange("p (h c) -> p h c", c=DH + 1)
            # fold the mask into V and the ones column: masked m-rows
            # contribute 0 to both the numerator and the softmax sum
            nc.vector.tensor_scalar_mul(
                dst[:, :, 0:DH],
                ps[:].rearrange("p (h c) -> p h c", c=DH),
                mask_t[:, t:t + 1])
            nc.vector.memset(dst[:, :, DH:DH + 1], 1.0)
            nc.vector.tensor_scalar_mul(dst[:, :, DH:DH + 1],
                                        dst[:, :, DH:DH + 1],
                                        mask_t[:, t:t + 1])
            yield

        def g_qchain(p, j):
            xt = xt_tiles[j]
            ps = aux_p.tile([PT, NB], F32, tag="aux", name="aux")
            for k in range(KT_DQ):
                nc.tensor.matmul(ps[:], wq_t[k][:, p * PT:(p + 1) * PT],
                                 xt[k],
                                 start=(k == 0), stop=(k == KT_DQ - 1))
                if k % 2 == 1:
                    yield
            nc.vector.tensor_copy(qt[p][:, j * NB:(j + 1) * NB], ps[:])
            yield

        def g_dchunk(j, nt):
            # out rows nt*128..(nt+1)*128  =  ot[:, nt-slice].T @ Wo
            ob = pdo.tile([PT, INNER], F32, tag="dout", name="dout")
            for c in range(INNER // NB):
                ps = aux_p.tile([PT, NB], F32, tag="aux", name="aux")
                for k in range(4):
                    nc.tensor.matmul(
                        ps[:], ot[k][:, nt * PT:(nt + 1) * PT],
                        wo_t[k][:, c * NB:(c + 1) * NB],
                        start=(k == 0), stop=(k == 3))
                    yield
                nc.vector.tensor_copy(ob[:, c * NB:(c + 1) * NB], ps[:])
            nc.sync.dma_start(out_d[nt * PT:(nt + 1) * PT, :], ob[:])
            yield

        # ---------------- attention emitters ----------------
        def emit_s_exp(p, j, t):
            jq = slice(j * NB, (j + 1) * NB)
            sps = sps_p.tile([PT, 2 * NB], F32, tag="sps", name="sps")
            nc.tensor.matmul(sps[:, 0:NB],
                             kt[p][0:DH, t * PT:(t + 1) * PT],
                             qt[p][0:DH, jq], start=True, stop=True)
            nc.tensor.matmul(sps[:, NB:2 * NB],
                             kt[p][DH:2 * DH, t * PT:(t + 1) * PT],
                             qt[p][DH:2 * DH, jq], start=True, stop=True)
            pe = ppe.tile([PT, 2 * NB], BF16, tag="pe", name="pe")
            nc.scalar.activation(pe[:], sps[:], EXP, scale=SCALE)
            return pe

        def emit_av(pes, oA, oB, hA, hB, t, starts=(0,), stops=(MT - 1,)):
            nc.tensor.matmul(oA[:],
                             vt[t][:, hA * (DH + 1):(hA + 1) * (DH + 1)],
                             pes[:, 0:NB],
                             start=(t in starts), stop=(t in stops))
            nc.tensor.matmul(oB[:],
                             vt[t][:, hB * (DH + 1):(hB + 1) * (DH + 1)],
                             pes[:, NB:2 * NB],
                             start=(t in starts), stop=(t in stops))

        stage_tiles = {}

        def emit_stage(prev):
            # j0 round A: bank the m 0..7 partial in SBUF, freeing oacc
            p, j, oA, oB = prev[:4]
            sA = pst.tile([DH + 1, NB], F32, tag=f"sA{p}", name=f"sA{p}")
            sB = pst.tile([DH + 1, NB], F32, tag=f"sB{p}", name=f"sB{p}")
            nc.vector.tensor_copy(sA[:], oA[:])
            nc.vector.tensor_copy(sB[:], oB[:])
            stage_tiles[p] = (sA, sB)

        def emit_normalize(prev, last=False):
            # stage oA/oB out to SBUF first: oacc has bufs=1, so the psum
            # must be free before the next window's first AV; everything
            # after the two staging copies is off the critical path
            p, j, oA, oB = prev[:4]
            merge = prev[4] if len(prev) > 4 else False
            jq = slice(j * NB, (j + 1) * NB)
            ocA = psm.tile([DH + 1, NB], F32, tag="ocA", name="ocA")
            ocB = psm.tile([DH + 1, NB], F32, tag="ocB", name="ocB")
            if merge:
                sA, sB = stage_tiles[p]
                nc.vector.tensor_add(ocA[:], oA[:], sA[:])
                nc.vector.tensor_add(ocB[:], oB[:], sB[:])
            else:
                nc.vector.tensor_copy(ocA[:], oA[:])
                nc.vector.tensor_copy(ocB[:], oB[:])
            sums = psm.tile([1, 2 * NB], F32, tag="sums", name="sums")
            nc.vector.tensor_copy(sums[0:1, 0:NB], ocA[DH:DH + 1, :])
            nc.vector.tensor_copy(sums[0:1, NB:2 * NB], ocB[DH:DH + 1, :])
            rr = psm.tile([1, 2 * NB], F32, tag="rr", name="rr")
            nc.vector.reciprocal_approx_fast(rr[0:1, :], sums[0:1, :])
            if last:
                # PE-matmul broadcast (ones64^T @ rr) keeps the PE warm into
                # phase D and skips the slow gpsimd chain at the tail
                rrb = psm.tile([1, 2 * NB], BF16, tag="rrb", name="rrb")
                nc.vector.tensor_copy(rrb[0:1, :], rr[0:1, :])
                bpA = aux_p.tile([PT, NB], F32, tag="aux", name="aux")
                bpB = aux_p.tile([PT, NB], F32, tag="aux", name="aux")
                nc.tensor.matmul(bpA[0:DH, :], ones64[0:1, :],
                                 rrb[0:1, 0:NB], start=True, stop=True)
                nc.tensor.matmul(bpB[0:DH, :], ones64[0:1, :],
                                 rrb[0:1, NB:2 * NB], start=True, stop=True)
                bA, bB = bpA[0:DH, 0:NB], bpB[0:DH, 0:NB]
            else:
                bcs = psm.tile([DH, 2 * NB], F32, tag="bcs", name="bcs")
                nc.gpsimd.partition_broadcast(bcs[:], rr[0:1, :])
                bA, bB = bcs[:, 0:NB], bcs[:, NB:2 * NB]
            nc.vector.tensor_mul(ot[p][0:DH, jq], ocA[0:DH, :], bA)
            tmpB = psm.tile([DH, NB], BF16, tag="tmpB", name="tmpB")
            nc.vector.tensor_mul(tmpB[:], ocB[0:DH, :], bB)
            nc.sync.dma_start(ot[p][DH:2 * DH, jq], tmpB[:])

        # ---------------- emission ----------------
        # DMAs: exp-critical first, one coalesced DMA per tensor (each
        # trigger costs ~650ns of SP-queue time, so fewer is faster)
        nc.vector.memset(ones64[0:1, :], 1.0)
        nc.sync.dma_start(wk_all[:], wk_d[:, :])
        dma_ctx(0)
        nc.sync.dma_start(wq_all[:], wq_d[:, :])
        dma_x(0)
        nc.sync.dma_start(mask_t[:], mb_d[:, :])
        nc.sync.dma_start(wv_all[:], wv_d[:, :])
        nc.sync.dma_start(wo_all[:], wo_d[:, :])

        # prologue: just enough for (j0, p0, t=0..3)
        wq.add(("ktq", 0, 0), g_ktq(0, 0))
        wq.add(("qt", 0, 0), g_qchain(0, 0))
        for t in range(4):
            wq.add(("vt", t), g_vt(t))
        # j0 runs in two m-rounds; order W by when each chain is consumed:
        # round A (m 0..7): p0 needs ktq(0,0..1)+vt(0..7); p1-3 their ktq/qt
        # round B (m 8..15): p0's second half, then p1-3's
        wq.add(("dma_ctx", 1), g_fn(dma_ctx, 1))
        wq.add(("ktq", 0, 1), g_ktq(0, 1))
        for t in range(4, 8):
            wq.add(("vt", t), g_vt(t))
        for p in range(1, 4):
            wq.add(("ktq", p, 0), g_ktq(p, 0))
            wq.add(("ktq", p, 1), g_ktq(p, 1))
            wq.add(("qt", p, 0), g_qchain(p, 0))
        for q in range(2, 4):
            wq.add(("dma_ctx", q), g_fn(dma_ctx, q))
            wq.add(("ktq", 0, q), g_ktq(0, q))
            for t in range(4 * q, 4 * q + 4):
                wq.add(("vt", t), g_vt(t))
        for p in range(1, 4):
            wq.add(("ktq", p, 2), g_ktq(p, 2))
            wq.add(("ktq", p, 3), g_ktq(p, 3))
        # Q chains for j1..3 (x DMA ahead of each group)
        for j in range(1, NJ):
            wq.add(("dma_x", j), g_fn(dma_x, j))
            for p in range(4):
                wq.add(("qt", p, j), g_qchain(p, j))

        prev = None

        def finish_prev():
            nonlocal prev
            if prev is None:
                return
            if prev[4] == "stage":
                emit_stage(prev)
            else:
                emit_normalize(prev)
            prev = None

        for j in range(NJ):
            rounds = ([(range(0, 8), "stage"), (range(8, MT), "merge")]
                      if j == 0 else [(range(MT), "norm")])
            for ts, kind in rounds:
                for p in range(4):
                    hA, hB = 2 * p, 2 * p + 1
                    wq.drain(("qt", p, j))
                    oA = oacc_p.tile([DH + 1, NB], F32, tag="oA", name="oA")
                    oB = oacc_p.tile([DH + 1, NB], F32, tag="oB", name="oB")
                    pes = {}
                    t0r = ts[0]
                    for t in ts:
                        if j == 0:
                            wq.drain(("ktq", p, t // 4))
                            if p == 0:
                                wq.drain(("vt", t))
                        pes[t] = emit_s_exp(p, j, t)
                        # oacc has bufs=1: the previous window's reader
                        # (stage/normalize) must be emitted before this
                        # window's first AV (the overwriter) lands at t0r+1
                        if t == t0r:
                            finish_prev()
                        if t == 3 and p == 0 and j > 0:
                            # ot[*][:, (j-1)-block] all normalized now
                            for nt in range(4 * (j - 1), 4 * j):
                                wq.add(("D", nt), g_dchunk(j - 1, nt))
                        if t > t0r:
                            emit_av(pes[t - 1], oA, oB, hA, hB, t - 1,
                                    starts=(t0r,), stops=(ts[-1],))
                            pes[t - 1] = None
                        wq.pump(1)
                    emit_av(pes[ts[-1]], oA, oB, hA, hB, ts[-1],
                            starts=(t0r,), stops=(ts[-1],))
                    if kind == "stage":
                        prev = (p, j, oA, oB, "stage")
                    else:
                        prev = (p, j, oA, oB, kind == "merge")
        emit_normalize(prev, last=True)
        for nt in range(4 * (NJ - 1), 4 * NJ):
            wq.add(("D", nt), g_dchunk(NJ - 1, nt))
        wq.drain_all()

    nc.compile()
    return nc


def _get_nc():
    if "nc" not in _CACHE:
        _CACHE["nc"] = _build_nc()
    return _CACHE["nc"]


def make_in_maps(x, context, mask, Wq, Wk, Wv, Wo):
    import ml_dtypes
    bf16 = ml_dtypes.bfloat16
    x = np.asarray(x, np.float32)
    context = np.asarray(context, np.float32)
    mask = np.asarray(mask)
    mask01 = np.where(mask, np.float32(1.0), np.float32(0.0))
    def chunk_rows(a, kt):
        # [kt*128, F] -> [128, kt*F]: row k*128+p lands at [p, k*F:...]
        r, f = a.shape
        return np.ascontiguousarray(
            a.reshape(kt, PT, f).transpose(1, 0, 2).reshape(PT, kt * f))

    def quarters(aT, nq):
        # [1024, nq*512] -> [128, nq*8*512] quarter-major
        return np.ascontiguousarray(
            aT.reshape(KT_DQ, PT, nq, NB).transpose(1, 2, 0, 3)
            .reshape(PT, nq * KT_DQ * NB))

    wqs, wks, wvs, wos = [], [], [], []
    for g in range(HG):
        cs = slice(g * CI, (g + 1) * CI)
        wqs.append(chunk_rows(np.asarray(Wq, np.float32)[:, cs].astype(bf16),
                              KT_DQ))
        wks.append(chunk_rows(np.asarray(Wk, np.float32)[:, cs].astype(bf16),
                              KT_DQ))
        wvs.append(chunk_rows(np.asarray(Wv, np.float32)[:, cs].astype(bf16),
                              KT_DQ))
        wos.append(chunk_rows(np.asarray(Wo, np.float32)[cs, :].astype(bf16),
                              4))
    in_maps = []
    for b in range(B):
        xT = quarters(x[b].T.astype(bf16), NJ)
        ctxT = quarters(context[b].T.astype(bf16), 4)
        mb = np.ascontiguousarray(mask01[b].reshape(MT, PT).T)
        for g in range(HG):
            in_maps.append({
                "xT": xT, "ctxT": ctxT,
                "wq": wqs[g], "wk": wks[g], "wv": wvs[g], "wo": wos[g],
                "mask01": mb,
            })
    return in_maps


def combine(results, bo):
    bo = np.asarray(bo, np.float32)
    out = np.empty((B, N, INNER), np.float32)
    for b in range(B):
        out[b] = (results[2 * b]["out"] + results[2 * b + 1]["out"]
                  + bo[None, :])
    return out


def kernel(x, context, mask, Wq, Wk, Wv, Wo, bo):
    from concourse import bass2jax
    nc = _get_nc()
    in_maps = make_in_maps(x, context, mask, Wq, Wk, Wv, Wo)
    results = bass2jax.run_bass_via_pjrt(nc, in_maps, n_cores=NCORES)
    return combine(results, bo)


# revision 11
# speedup vs baseline: 1.0021x; 1.0021x over previous
"""Fused cross-attention kernel for TRN2, sharded over 8 NeuronCores.

Sharding: core = 2*b + g  (b = batch 0..3 data-parallel, g = head-group 0..1
tensor-parallel over heads: heads g*8..g*8+7, i.e. columns g*512..(g+1)*512 of
Wq/Wk/Wv and rows g*512..(g+1)*512 of Wo). Each core computes a partial
out = softmax((x@Wq)(ctx@Wk)^T/sqrt(d)) (ctx@Wv) @ Wo_slice for its batch;
the host sums the two head-group partials per batch and adds bo.

Schedule: the ScalarE exp stream (256 x [128,1024] activations ~ 285us) is
the critical resource; everything else hides under it.  Loops run j (n-block)
outer, pair inner, m-tile innermost.  All non-attention PE work (K^T/V/Q
projections, out = O^T.T@Wo) is emitted through a work queue that drips ~1
matmul per t-step into the PE queue, plus watermark draining so the first
j-block can start ~15us in while K/V production continues underneath.
S pairs are row-tiled (heads at PE row tiles 0/64) and stream concurrently.
Mask is folded into V and the ones-column (masked rows contribute 0 to both
numerator and softmax sum), so exp needs no bias operand.  Normalize uses
DVE + a 0-stride DMA partition-broadcast (no gpsimd).
"""
import numpy as np

B, N, M = 4, 2048, 2048
DQ = 1024
DC = 1024
H = 16
DH = 64
INNER = 1024
HG = 2            # head groups (tensor parallel)
HPC = H // HG     # heads per core
CI = HPC * DH     # 512 inner dims per core
NCORES = 8
PT = 128          # partition tile
NB = 512          # n-block
KT_DQ = DQ // PT  # 8 contraction tiles for projections
MT = M // PT      # 16 m-tiles
NT = N // PT      # 16 n-tiles
NJ = N // NB      # 4 n-blocks
SCALE = DH ** -0.5

_CACHE = {}


class WorkQueue:
    """Ordered generators of background PE work, dripped into the emission
    stream.  pump(n) advances n yield-units; drain(tag) runs until the
    generator registered under tag has completed."""

    def __init__(self):
        self.items = []      # list of (tag, generator)
        self.done = set()
        self.active = None   # (tag, gen)

    def add(self, tag, gen):
        self.items.append((tag, gen))

    def _step(self):
        # advance the current generator by one unit; True if work remains
        if self.active is None:
            if not self.items:
                return False
            self.active = self.items.pop(0)
        tag, gen = self.active
        try:
            next(gen)
        except StopIteration:
            self.done.add(tag)
            self.active = None
        return True

    def pump(self, n):
        for _ in range(n):
            if not self._step():
                return

    def drain(self, tag):
        while tag not in self.done:
            if not self._step():
                raise RuntimeError(f"work item {tag} never registered")

    def drain_all(self):
        while self._step():
            pass


def _build_nc():
    import concourse.bass as bass
    import concourse.mybir as mybir
    import concourse.tile as tile
    from concourse import bacc

    F32 = mybir.dt.float32
    BF16 = mybir.dt.bfloat16
    EXP = mybir.ActivationFunctionType.Exp

    nc = bacc.Bacc("TRN2", target_bir_lowering=False, debug=False,
                   num_devices=NCORES)

    # host-prearranged: [128, ...] so every load is a 2D contiguous DMA
    xT_d = nc.dram_tensor("xT", [PT, NJ * KT_DQ * NB], BF16,
                          kind="ExternalInput")
    ctxT_d = nc.dram_tensor("ctxT", [PT, 4 * KT_DQ * NB], BF16,
                            kind="ExternalInput")
    wq_d = nc.dram_tensor("wq", [PT, KT_DQ * CI], BF16, kind="ExternalInput")
    wk_d = nc.dram_tensor("wk", [PT, KT_DQ * CI], BF16, kind="ExternalInput")
    wv_d = nc.dram_tensor("wv", [PT, KT_DQ * CI], BF16, kind="ExternalInput")
    wo_d = nc.dram_tensor("wo", [PT, 4 * INNER], BF16, kind="ExternalInput")
    mb_d = nc.dram_tensor("mask01", [PT, MT], F32, kind="ExternalInput")
    out_d = nc.dram_tensor("out", [N, INNER], F32, kind="ExternalOutput")

    with tile.TileContext(nc) as tc:
      with (
          tc.tile_pool(name="persist", bufs=1) as pp,
          tc.tile_pool(name="ctx_s", bufs=3) as pcs,
          tc.tile_pool(name="xt_s", bufs=2) as pxs,
          tc.tile_pool(name="pe_p", bufs=3) as ppe,
          tc.tile_pool(name="small", bufs=2) as psm,
          tc.tile_pool(name="stg", bufs=1) as pst,
          tc.tile_pool(name="dout", bufs=2) as pdo,
          tc.tile_pool(name="sps_p", bufs=2, space="PSUM") as sps_p,
          tc.tile_pool(name="oacc", bufs=1, space="PSUM") as oacc_p,
          tc.tile_pool(name="aux", bufs=2, space="PSUM") as aux_p,
      ):
        kt = [pp.tile([PT, M], BF16, tag=f"kt{p}", name=f"kt{p}")
              for p in range(4)]
        vt = [pp.tile([PT, HPC * (DH + 1)], BF16, tag=f"vt{t}", name=f"vt{t}")
              for t in range(MT)]
        qt = [pp.tile([PT, N], BF16, tag=f"qt{p}", name=f"qt{p}")
              for p in range(4)]
        ot = [pp.tile([PT, N], BF16, tag=f"ot{p}", name=f"ot{p}")
              for p in range(4)]
        mask_t = pp.tile([PT, MT], F32, tag="mask", name="mask")
        ones64 = pp.tile([1, DH], BF16, tag="ones64", name="ones64")
        wq_all = pp.tile([PT, KT_DQ * CI], BF16, tag="wq_all", name="wq_all")
        wk_all = pp.tile([PT, KT_DQ * CI], BF16, tag="wk_all", name="wk_all")
        wv_all = pp.tile([PT, KT_DQ * CI], BF16, tag="wv_all", name="wv_all")
        wo_all = pp.tile([PT, 4 * INNER], BF16, tag="wo_all", name="wo_all")
        wq_t = [wq_all[:, k * CI:(k + 1) * CI] for k in range(KT_DQ)]
        wk_t = [wk_all[:, k * CI:(k + 1) * CI] for k in range(KT_DQ)]
        wv_t = [wv_all[:, k * CI:(k + 1) * CI] for k in range(KT_DQ)]
        wo_t = [wo_all[:, k * INNER:(k + 1) * INNER] for k in range(4)]

        wq = WorkQueue()

        # ---------------- staging DMA helpers ----------------
        ctx_tiles = {}   # q -> list of 8 tiles

        def dma_ctx(q):
            ca = pcs.tile([PT, KT_DQ * NB], BF16, tag="ctxq", name="ctxq")
            nc.sync.dma_start(
                ca[:], ctxT_d[:, q * KT_DQ * NB:(q + 1) * KT_DQ * NB])
            ctx_tiles[q] = [ca[:, k * NB:(k + 1) * NB] for k in range(KT_DQ)]

        xt_tiles = {}    # j -> list of 8 tiles

        def dma_x(j):
            xa = pxs.tile([PT, KT_DQ * NB], BF16, tag="xq", name="xq")
            nc.sync.dma_start(
                xa[:], xT_d[:, j * KT_DQ * NB:(j + 1) * KT_DQ * NB])
            xt_tiles[j] = [xa[:, k * NB:(k + 1) * NB] for k in range(KT_DQ)]

        # ---------------- background work generators ----------------
        def g_fn(fn, *a):
            def g():
                fn(*a)
                yield
            return g()

        def g_ktq(p, q):
            ctx = ctx_tiles[q]
            ps = aux_p.tile([PT, NB], F32, tag="aux", name="aux")
            for k in range(KT_DQ):
                nc.tensor.matmul(ps[:], wk_t[k][:, p * PT:(p + 1) * PT],
                                 ctx[k],
                                 start=(k == 0), stop=(k == KT_DQ - 1))
                if k % 2 == 1:
                    yield
            nc.vector.tensor_copy(kt[p][:, q * NB:(q + 1) * NB], ps[:])
            yield

        def g_vt(t):
            q = t // 4
            ti = t % 4
            ctx = ctx_tiles[q]
            ps = aux_p.tile([PT, CI], F32, tag="aux", name="aux")
            for k in range(KT_DQ):
                nc.tensor.matmul(ps[:], ctx[k][:, ti * PT:(ti + 1) * PT],
                                 wv_t[k][:],
                                 start=(k == 0), stop=(k == KT_DQ - 1))
                if k % 2 == 1:
                    yield
            dst = vt[t][:].rearrange("p (h c) -> p h c", c=DH + 1)
            # fold the mask into V and the ones column: masked m-rows
            # contribute 0 to both the numerator and the softmax sum
            nc.vector.tensor_scalar_mul(
                dst[:, :, 0:DH],
                ps[:].rearrange("p (h c) -> p h c", c=DH),
                mask_t[:, t:t + 1])
            nc.vector.memset(dst[:, :, DH:DH + 1], 1.0)
            nc.vector.tensor_scalar_mul(dst[:, :, DH:DH + 1],
                                        dst[:, :, DH:DH + 1],
                                        mask_t[:, t:t + 1])
            yield

        def g_qchain(p, j):
            xt = xt_tiles[j]
            ps = aux_p.tile([PT, NB], F32, tag="aux", name="aux")
            for k in range(KT_DQ):
                nc.tensor.matmul(ps[:], wq_t[k][:, p * PT:(p + 1) * PT],
                                 xt[k],
                                 start=(k == 0), stop=(k == KT_DQ - 1))
                if k % 2 == 1:
                    yield
            nc.vector.tensor_copy(qt[p][:, j * NB:(j + 1) * NB], ps[:])
            yield

        def g_dchunk(j, nt):
            # out rows nt*128..(nt+1)*128  =  ot[:, nt-slice].T @ Wo
            ob = pdo.tile([PT, INNER], F32, tag="dout", name="dout")
            for c in range(INNER // NB):
                ps = aux_p.tile([PT, NB], F32, tag="aux", name="aux")
                for k in range(4):
                    nc.tensor.matmul(
                        ps[:], ot[k][:, nt * PT:(nt + 1) * PT],
                        wo_t[k][:, c * NB:(c + 1) * NB],
                        start=(k == 0), stop=(k == 3))
                    yield
                nc.vector.tensor_copy(ob[:, c * NB:(c + 1) * NB], ps[:])
            nc.sync.dma_start(out_d[nt * PT:(nt + 1) * PT, :], ob[:])
            yield

        # ---------------- attention emitters ----------------
        def emit_s_exp(p, j, t):
            jq = slice(j * NB, (j + 1) * NB)
            sps = sps_p.tile([PT, 2 * NB], F32, tag="sps", name="sps")
            nc.tensor.matmul(sps[:, 0:NB],
                             kt[p][0:DH, t * PT:(t + 1) * PT],
                             qt[p][0:DH, jq], start=True, stop=True)
            nc.tensor.matmul(sps[:, NB:2 * NB],
                             kt[p][DH:2 * DH, t * PT:(t + 1) * PT],
                             qt[p][DH:2 * DH, jq], start=True, stop=True)
            pe = ppe.tile([PT, 2 * NB], BF16, tag="pe", name="pe")
            nc.scalar.activation(pe[:], sps[:], EXP, scale=SCALE)
            return pe

        def emit_av(pes, oA, oB, hA, hB, t, starts=(0,), stops=(MT - 1,)):
            nc.tensor.matmul(oA[:],
                             vt[t][:, hA * (DH + 1):(hA + 1) * (DH + 1)],
                             pes[:, 0:NB],
                             start=(t in starts), stop=(t in stops))
            nc.tensor.matmul(oB[:],
                             vt[t][:, hB * (DH + 1):(hB + 1) * (DH + 1)],
                             pes[:, NB:2 * NB],
                             start=(t in starts), stop=(t in stops))

        stage_tiles = {}

        def emit_stage(prev):
            # j0 round A: bank the m 0..7 partial in SBUF, freeing oacc
            p, j, oA, oB = prev[:4]
            sA = pst.tile([DH + 1, NB], F32, tag=f"sA{p}", name=f"sA{p}")
            sB = pst.tile([DH + 1, NB], F32, tag=f"sB{p}", name=f"sB{p}")
            nc.vector.tensor_copy(sA[:], oA[:])
            nc.vector.tensor_copy(sB[:], oB[:])
            stage_tiles[p] = (sA, sB)

        def emit_normalize(prev, last=False):
            # stage oA/oB out to SBUF first: oacc has bufs=1, so the psum
            # must be free before the next window's first AV; everything
            # after the two staging copies is off the critical path
            p, j, oA, oB = prev[:4]
            merge = prev[4] if len(prev) > 4 else False
            jq = slice(j * NB, (j + 1) * NB)
            ocA = psm.tile([DH + 1, NB], F32, tag="ocA", name="ocA")
            ocB = psm.tile([DH + 1, NB], F32, tag="ocB", name="ocB")
            if merge:
                sA, sB = stage_tiles[p]
                nc.vector.tensor_add(ocA[:], oA[:], sA[:])
                nc.vector.tensor_add(ocB[:], oB[:], sB[:])
            else:
                nc.vector.tensor_copy(ocA[:], oA[:])
                nc.vector.tensor_copy(ocB[:], oB[:])
            if last:
                # per-head-independent chains (shorter latency) + PE-matmul
                # broadcast: keeps the PE warm into phase D and skips gpsimd
                sums = psm.tile([1, 2 * NB], F32, tag="sums", name="sums")
                rr = psm.tile([1, 2 * NB], F32, tag="rr", name="rr")
                rrb = psm.tile([1, 2 * NB], BF16, tag="rrb", name="rrb")
                bpA = aux_p.tile([PT, NB], F32, tag="aux", name="aux")
                bpB = aux_p.tile([PT, NB], F32, tag="aux", name="aux")
                tmpB = psm.tile([DH, NB], BF16, tag="tmpB", name="tmpB")
                nc.vector.tensor_copy(sums[0:1, 0:NB], ocA[DH:DH + 1, :])
                nc.vector.reciprocal_approx_fast(rr[0:1, 0:NB],
                                                 sums[0:1, 0:NB])
                nc.vector.tensor_copy(rrb[0:1, 0:NB], rr[0:1, 0:NB])
                nc.tensor.matmul(bpA[0:DH, :], ones64[0:1, :],
                                 rrb[0:1, 0:NB], start=True, stop=True)
                nc.vector.tensor_mul(ot[p][0:DH, jq], ocA[0:DH, :],
                                     bpA[0:DH, 0:NB])
                nc.vector.tensor_copy(sums[0:1, NB:2 * NB], ocB[DH:DH + 1, :])
                nc.vector.reciprocal_approx_fast(rr[0:1, NB:2 * NB],
                                                 sums[0:1, NB:2 * NB])
                nc.vector.tensor_copy(rrb[0:1, NB:2 * NB],
                                      rr[0:1, NB:2 * NB])
                nc.tensor.matmul(bpB[0:DH, :], ones64[0:1, :],
                                 rrb[0:1, NB:2 * NB], start=True, stop=True)
                nc.vector.tensor_mul(tmpB[:], ocB[0:DH, :], bpB[0:DH, 0:NB])
                nc.sync.dma_start(ot[p][DH:2 * DH, jq], tmpB[:])
                return
            sums = psm.tile([1, 2 * NB], F32, tag="sums", name="sums")
            nc.vector.tensor_copy(sums[0:1, 0:NB], ocA[DH:DH + 1, :])
            nc.vector.tensor_copy(sums[0:1, NB:2 * NB], ocB[DH:DH + 1, :])
            rr = psm.tile([1, 2 * NB], F32, tag="rr", name="rr")
            nc.vector.reciprocal_approx_fast(rr[0:1, :], sums[0:1, :])
            bcs = psm.tile([DH, 2 * NB], F32, tag="bcs", name="bcs")
            nc.gpsimd.partition_broadcast(bcs[:], rr[0:1, :])
            nc.vector.tensor_mul(ot[p][0:DH, jq], ocA[0:DH, :], bcs[:, 0:NB])
            tmpB = psm.tile([DH, NB], BF16, tag="tmpB", name="tmpB")
            nc.vector.tensor_mul(tmpB[:], ocB[0:DH, :], bcs[:, NB:2 * NB])
            nc.sync.dma_start(ot[p][DH:2 * DH, jq], tmpB[:])

        # ---------------- emission ----------------
        # DMAs: exp-critical first, one coalesced DMA per tensor (each
        # trigger costs ~650ns of SP-queue time, so fewer is faster)
        nc.vector.memset(ones64[0:1, :], 1.0)
        nc.sync.dma_start(wq_all[:], wq_d[:, :])
        dma_x(0)
        nc.sync.dma_start(wk_all[:], wk_d[:, :])
        dma_ctx(0)
        nc.sync.dma_start(mask_t[:], mb_d[:, :])
        nc.sync.dma_start(wv_all[:], wv_d[:, :])
        nc.sync.dma_start(wo_all[:], wo_d[:, :])

        # prologue: just enough for (j0, p0, t=0..3)
        wq.add(("qt", 0, 0), g_qchain(0, 0))
        wq.add(("ktq", 0, 0), g_ktq(0, 0))
        for t in range(4):
            wq.add(("vt", t), g_vt(t))
        # j0 runs in two m-rounds; order W by when each chain is consumed:
        # round A (m 0..7): p0 needs ktq(0,0..1)+vt(0..7); p1-3 their ktq/qt
        # round B (m 8..15): p0's second half, then p1-3's
        wq.add(("dma_ctx", 1), g_fn(dma_ctx, 1))
        wq.add(("ktq", 0, 1), g_ktq(0, 1))
        for t in range(4, 8):
            wq.add(("vt", t), g_vt(t))
        for p in range(1, 4):
            wq.add(("ktq", p, 0), g_ktq(p, 0))
            wq.add(("ktq", p, 1), g_ktq(p, 1))
            wq.add(("qt", p, 0), g_qchain(p, 0))
        for q in range(2, 4):
            wq.add(("dma_ctx", q), g_fn(dma_ctx, q))
            wq.add(("ktq", 0, q), g_ktq(0, q))
            for t in range(4 * q, 4 * q + 4):
                wq.add(("vt", t), g_vt(t))
        for p in range(1, 4):
            wq.add(("ktq", p, 2), g_ktq(p, 2))
            wq.add(("ktq", p, 3), g_ktq(p, 3))
        # Q chains for j1..3 (x DMA ahead of each group)
        for j in range(1, NJ):
            wq.add(("dma_x", j), g_fn(dma_x, j))
            for p in range(4):
                wq.add(("qt", p, j), g_qchain(p, j))

        prev = None

        def finish_prev():
            nonlocal prev
            if prev is None:
                return
            if prev[4] == "stage":
                emit_stage(prev)
            else:
                emit_normalize(prev)
            prev = None

        for j in range(NJ):
            rounds = ([(range(0, 8), "stage"), (range(8, MT), "merge")]
                      if j == 0 else [(range(MT), "norm")])
            for ts, kind in rounds:
                for p in range(4):
                    hA, hB = 2 * p, 2 * p + 1
                    wq.drain(("qt", p, j))
                    oA = oacc_p.tile([DH + 1, NB], F32, tag="oA", name="oA")
                    oB = oacc_p.tile([DH + 1, NB], F32, tag="oB", name="oB")
                    pes = {}
                    t0r = ts[0]
                    for t in ts:
                        if j == 0:
                            wq.drain(("ktq", p, t // 4))
                            if p == 0:
                                wq.drain(("vt", t))
                        pes[t] = emit_s_exp(p, j, t)
                        # oacc has bufs=1: the previous window's reader
                        # (stage/normalize) must be emitted before this
                        # window's first AV (the overwriter) lands at t0r+1
                        if t == t0r:
                            finish_prev()
                        if t == 3 and p == 0 and j > 0:
                            # ot[*][:, (j-1)-block] all normalized now
                            for nt in range(4 * (j - 1), 4 * j):
                                wq.add(("D", nt), g_dchunk(j - 1, nt))
                        if t > t0r:
                            emit_av(pes[t - 1], oA, oB, hA, hB, t - 1,
                                    starts=(t0r,), stops=(ts[-1],))
                            pes[t - 1] = None
                        wq.pump(1)
                    emit_av(pes[ts[-1]], oA, oB, hA, hB, ts[-1],
                            starts=(t0r,), stops=(ts[-1],))
                    if kind == "stage":
                        prev = (p, j, oA, oB, "stage")
                    else:
                        prev = (p, j, oA, oB, kind == "merge")
        emit_normalize(prev, last=True)
        for nt in range(4 * (NJ - 1), 4 * NJ):
            wq.add(("D", nt), g_dchunk(NJ - 1, nt))
        wq.drain_all()

    nc.compile()
    return nc


def _get_nc():
    if "nc" not in _CACHE:
        _CACHE["nc"] = _build_nc()
    return _CACHE["nc"]


def make_in_maps(x, context, mask, Wq, Wk, Wv, Wo):
    import ml_dtypes
    bf16 = ml_dtypes.bfloat16
    x = np.asarray(x, np.float32)
    context = np.asarray(context, np.float32)
    mask = np.asarray(mask)
    mask01 = np.where(mask, np.float32(1.0), np.float32(0.0))
    def chunk_rows(a, kt):
        # [kt*128, F] -> [128, kt*F]: row k*128+p lands at [p, k*F:...]
        r, f = a.shape
        return np.ascontiguousarray(
            a.reshape(kt, PT, f).transpose(1, 0, 2).reshape(PT, kt * f))

    def quarters(aT, nq):
        # [1024, nq*512] -> [128, nq*8*512] quarter-major
        return np.ascontiguousarray(
            aT.reshape(KT_DQ, PT, nq, NB).transpose(1, 2, 0, 3)
            .reshape(PT, nq * KT_DQ * NB))

    wqs, wks, wvs, wos = [], [], [], []
    for g in range(HG):
        cs = slice(g * CI, (g + 1) * CI)
        wqs.append(chunk_rows(np.asarray(Wq, np.float32)[:, cs].astype(bf16),
                              KT_DQ))
        wks.append(chunk_rows(np.asarray(Wk, np.float32)[:, cs].astype(bf16),
                              KT_DQ))
        wvs.append(chunk_rows(np.asarray(Wv, np.float32)[:, cs].astype(bf16),
                              KT_DQ))
        wos.append(chunk_rows(np.asarray(Wo, np.float32)[cs, :].astype(bf16),
                              4))
    in_maps = []
    for b in range(B):
        xT = quarters(x[b].T.astype(bf16), NJ)
        ctxT = quarters(context[b].T.astype(bf16), 4)
        mb = np.ascontiguousarray(mask01[b].reshape(MT, PT).T)
        for g in range(HG):
            in_maps.append({
                "xT": xT, "ctxT": ctxT,
                "wq": wqs[g], "wk": wks[g], "wv": wvs[g], "wo": wos[g],
                "mask01": mb,
            })
    return in_maps


def combine(results, bo):
    bo = np.asarray(bo, np.float32)
    out = np.empty((B, N, INNER), np.float32)
    for b in range(B):
        out[b] = (results[2 * b]["out"] + results[2 * b + 1]["out"]
                  + bo[None, :])
    return out


def kernel(x, context, mask, Wq, Wk, Wv, Wo, bo):
    from concourse import bass2jax
    nc = _get_nc()
    in_maps = make_in_maps(x, context, mask, Wq, Wk, Wv, Wo)
    results = bass2jax.run_bass_via_pjrt(nc, in_maps, n_cores=NCORES)
    return combine(results, bo)
